# revision 7
# baseline (speedup 1.0000x reference)
"""256-point FFT (real/imag channels) as a DFT matmul on Trainium2.

Contract: kernel(x) takes the FULL input x [131072, 2, 256] float32 and
returns the FULL output [131072, 2, 256] float32, computing, per batch row,
the 256-point complex FFT of (x[b,0,:] + i*x[b,1,:]) -> [real; imag].

Strategy (pure data parallel over 8 NeuronCores, 16384 rows/core):
  - Flatten each row to v[512] = [re(256), im(256)].  The FFT is the linear
    map y = v @ W with W[512,512] built from cos/sin of the DFT twiddles.
  - Per core, stream 128-row chunks: DMA rows in natural (batch-major)
    layout, transpose each [128b x 128n] block on TensorE (transpose mode,
    via identity), copy back to SBUF, then 4 accumulating matmuls against
    W (bf16 data/weights, fp32 PSUM accumulation) produce [128b x 512out]
    batch-major, which is copied to SBUF (ScalarE) and DMA'd out.
  - bf16 inputs + fp32 accumulate gives ~2e-3 relative error vs the fp32
    reference; DMA (64 MiB/core of HBM traffic) is the roofline.
"""

import numpy as np

B_TOTAL = 131072
N_CORES = 8
B_CORE = B_TOTAL // N_CORES  # 16384
NFFT = 256
V = 2 * NFFT  # 512, flattened row length
P = 128  # partitions

_cache = {}


def _dft_matrix_f64():
    """W[n, m] such that out[b, m] = sum_n v[b, n] * W[n, m]."""
    k = np.arange(NFFT, dtype=np.float64)
    theta = -2.0 * np.pi * np.outer(k, k) / NFFT  # [k, n]
    c = np.cos(theta).T  # [n, k]
    s = np.sin(theta).T  # [n, k]
    w = np.zeros((V, V), np.float64)
    w[:NFFT, :NFFT] = c
    w[NFFT:, :NFFT] = -s
    w[:NFFT, NFFT:] = s
    w[NFFT:, NFFT:] = c
    return w


def _build(b_core, super_rows, variant="bf16"):
    """Build + compile the per-core Bass program. Returns nc.

    variant: "bf16" (cast input to bf16 during load; ~2e-3 rel err) or
    "f32r" (fp32-width data, PE fp32r decomposition; higher precision).
    """
    import concourse.bass as bass
    import concourse.tile as tile
    from concourse import bacc, mybir

    n_super = b_core // super_rows
    r_sub = super_rows // P  # 128-row sub-chunks per DMA super-chunk
    f32 = mybir.dt.float32
    cdt = mybir.dt.bfloat16 if variant == "bf16" else mybir.dt.float32r

    nc = bacc.Bacc(
        "TRN2",
        target_bir_lowering=False,
        debug=False,
        num_devices=N_CORES,
    )
    x_dt = f32 if variant == "bf16" else mybir.dt.float32r
    x_d = nc.dram_tensor("x_in", [b_core, V], x_dt, kind="ExternalInput")
    w_d = nc.dram_tensor("w_in", [V, V], cdt, kind="ExternalInput")
    id_d = nc.dram_tensor("id_in", [P, P], cdt, kind="ExternalInput")
    y_d = nc.dram_tensor("y_out", [b_core, V], f32, kind="ExternalOutput")

    with tile.TileContext(nc) as tc:
        with (
            tc.tile_pool(name="const", bufs=1) as cpool,
            tc.tile_pool(name="xin", bufs=3) as xpool,
            tc.tile_pool(name="xt", bufs=4) as xtpool,
            tc.tile_pool(name="yout", bufs=3) as ypool,
            tc.tile_pool(name="psumT", bufs=3, space="PSUM") as ptpool,
            tc.tile_pool(name="psumO", bufs=3, space="PSUM") as popool,
        ):
            w_sb = cpool.tile([P, 4, V], cdt)
            nc.sync.dma_start(w_sb[:], w_d.ap().rearrange("(j p) m -> p j m", p=P))
            id_sb = cpool.tile([P, P], cdt)
            nc.sync.dma_start(id_sb[:], id_d.ap())

            for t in range(n_super):
                # Map 8 *consecutive* DRAM rows to each partition so every
                # partition's slice of the transfer is one contiguous 16 KiB
                # run (vs 2 KiB descriptors with row-round-robin layout).
                xin = xpool.tile([P, r_sub, V], cdt)
                load_eng = nc.gpsimd if variant == "bf16" else nc.sync
                load_eng.dma_start(
                    xin[:],
                    x_d.ap()[t * super_rows : (t + 1) * super_rows, :].rearrange(
                        "(p r) e -> p r e", p=P
                    ),
                )
                yout = ypool.tile([P, r_sub, V], f32)
                for r in range(r_sub):
                    psum_t = ptpool.tile([P, V], cdt)
                    for j in range(4):
                        nc.tensor.transpose(
                            psum_t[:, j * P : (j + 1) * P],
                            xin[:, r, j * P : (j + 1) * P],
                            id_sb[:],
                        )
                    xt = xtpool.tile([P, V], cdt)
                    nc.vector.tensor_copy(xt[:], psum_t[:])
                    psum_o = popool.tile([P, V], f32)
                    for j in range(4):
                        nc.tensor.matmul(
                            psum_o[:],
                            xt[:, j * P : (j + 1) * P],
                            w_sb[:, j, :],
                            start=(j == 0),
                            stop=(j == 3),
                        )
                    nc.scalar.copy(yout[:, r, :], psum_o[:])
                nc.sync.dma_start(
                    y_d.ap()[t * super_rows : (t + 1) * super_rows, :].rearrange(
                        "(p r) e -> p r e", p=P
                    ),
                    yout[:],
                )

    nc.compile()
    return nc


VARIANT = "bf16"


def _get_program(variant):
    key = ("prog", B_CORE, 1024, variant)
    if key not in _cache:
        _cache[key] = _build(B_CORE, 1024, variant)
    return _cache[key]


def _input_consts(variant):
    import ml_dtypes

    key = ("consts", variant)
    if key not in _cache:
        wdt = ml_dtypes.bfloat16 if variant == "bf16" else np.float32
        w = _dft_matrix_f64().astype(wdt)
        ident = np.eye(P, dtype=wdt)
        _cache[key] = (w, ident)
    return _cache[key]


def _run(x, trace=False, trace_cores=None, variant=None):
    """x: [B_TOTAL, 2, 256] f32 -> (out [B_TOTAL, 2, 256] f32, results obj)."""
    from concourse import bass_utils

    variant = variant or VARIANT
    x = np.ascontiguousarray(np.asarray(x, dtype=np.float32)).reshape(B_TOTAL, V)
    w, ident = _input_consts(variant)
    nc = _get_program(variant)
    in_maps = [
        {
            "x_in": x[c * B_CORE : (c + 1) * B_CORE],
            "w_in": w,
            "id_in": ident,
        }
        for c in range(N_CORES)
    ]
    res = bass_utils.run_bass_kernel_spmd(
        nc,
        in_maps,
        core_ids=list(range(N_CORES)),
        trace=trace,
        trace_cores=trace_cores,
    )
    out = np.concatenate([res.results[c]["y_out"] for c in range(N_CORES)], axis=0)
    return out.reshape(B_TOTAL, 2, NFFT).astype(np.float32, copy=False), res


def kernel(x):
    out, _ = _run(x, trace=False)
    return out


# revision 8
# speedup vs baseline: 1.1658x; 1.1658x over previous
"""256-point FFT (real/imag channels) as a DFT matmul on Trainium2.

Contract: kernel(x) takes the FULL input x [131072, 2, 256] float32 and
returns the FULL output [131072, 2, 256] float32, computing, per batch row,
the 256-point complex FFT of (x[b,0,:] + i*x[b,1,:]) -> [real; imag].

Strategy (pure data parallel over 8 NeuronCores, 16384 rows/core):
  - Flatten each row to v[512] = [re(256), im(256)].  The FFT is the linear
    map y = v @ W with W[512,512] built from cos/sin of the DFT twiddles.
  - Per core, stream 128-row chunks: DMA rows in natural (batch-major)
    layout, transpose each [128b x 128n] block on TensorE (transpose mode,
    via identity), copy back to SBUF, then 4 accumulating matmuls against
    W (bf16 data/weights, fp32 PSUM accumulation) produce [128b x 512out]
    batch-major, which is copied to SBUF (ScalarE) and DMA'd out.
  - bf16 inputs + fp32 accumulate gives ~2e-3 relative error vs the fp32
    reference; DMA (64 MiB/core of HBM traffic) is the roofline.
"""

import numpy as np

B_TOTAL = 131072
N_CORES = 8
B_CORE = B_TOTAL // N_CORES  # 16384
NFFT = 256
V = 2 * NFFT  # 512, flattened row length
P = 128  # partitions

_cache = {}


def _dft_matrix_f64():
    """W[n, m] such that out[b, m] = sum_n v[b, n] * W[n, m]."""
    k = np.arange(NFFT, dtype=np.float64)
    theta = -2.0 * np.pi * np.outer(k, k) / NFFT  # [k, n]
    c = np.cos(theta).T  # [n, k]
    s = np.sin(theta).T  # [n, k]
    w = np.zeros((V, V), np.float64)
    w[:NFFT, :NFFT] = c
    w[NFFT:, :NFFT] = -s
    w[:NFFT, NFFT:] = s
    w[NFFT:, NFFT:] = c
    return w


def _build(b_core, super_rows, variant="bf16"):
    """Build + compile the per-core Bass program. Returns nc.

    variant: "bf16" (cast input to bf16 during load; ~2e-3 rel err) or
    "f32r" (fp32-width data, PE fp32r decomposition; higher precision).
    """
    import concourse.bass as bass
    import concourse.tile as tile
    from concourse import bacc, mybir

    n_super = b_core // super_rows
    r_sub = super_rows // P  # 128-row sub-chunks per DMA super-chunk
    f32 = mybir.dt.float32
    cdt = mybir.dt.bfloat16 if variant == "bf16" else mybir.dt.float32r

    nc = bacc.Bacc(
        "TRN2",
        target_bir_lowering=False,
        debug=False,
        num_devices=N_CORES,
    )
    x_dt = f32 if variant == "bf16" else mybir.dt.float32r
    x_d = nc.dram_tensor("x_in", [b_core, V], x_dt, kind="ExternalInput")
    w_d = nc.dram_tensor("w_in", [V, V], cdt, kind="ExternalInput")
    id_d = nc.dram_tensor("id_in", [P, P], cdt, kind="ExternalInput")
    y_d = nc.dram_tensor("y_out", [b_core, V], f32, kind="ExternalOutput")

    with tile.TileContext(nc) as tc:
        with (
            tc.tile_pool(name="const", bufs=1) as cpool,
            tc.tile_pool(name="xin", bufs=3) as xpool,
            tc.tile_pool(name="xt", bufs=4) as xtpool,
            tc.tile_pool(name="yout", bufs=3) as ypool,
            tc.tile_pool(name="psumT", bufs=3, space="PSUM") as ptpool,
            tc.tile_pool(name="psumO", bufs=3, space="PSUM") as popool,
        ):
            w_sb = cpool.tile([P, 4, V], cdt)
            nc.sync.dma_start(w_sb[:], w_d.ap().rearrange("(j p) m -> p j m", p=P))
            id_sb = cpool.tile([P, P], cdt)
            nc.sync.dma_start(id_sb[:], id_d.ap())

            for t in range(n_super):
                # Map 8 *consecutive* DRAM rows to each partition so every
                # partition's slice of the transfer is one contiguous 16 KiB
                # run (vs 2 KiB descriptors with row-round-robin layout).
                xin = xpool.tile([P, r_sub, V], cdt)
                load_eng = nc.gpsimd if variant == "bf16" else nc.sync
                load_eng.dma_start(
                    xin[:],
                    x_d.ap()[t * super_rows : (t + 1) * super_rows, :].rearrange(
                        "(p r) e -> p r e", p=P
                    ),
                )
                yout = ypool.tile([P, r_sub, V], f32)
                for r in range(r_sub):
                    psum_t = ptpool.tile([P, V], cdt)
                    for j in range(4):
                        nc.tensor.transpose(
                            psum_t[:, j * P : (j + 1) * P],
                            xin[:, r, j * P : (j + 1) * P],
                            id_sb[:],
                        )
                    xt = xtpool.tile([P, V], cdt)
                    nc.vector.tensor_copy(xt[:], psum_t[:])
                    psum_o = popool.tile([P, V], f32)
                    for j in range(4):
                        nc.tensor.matmul(
                            psum_o[:],
                            xt[:, j * P : (j + 1) * P],
                            w_sb[:, j, :],
                            start=(j == 0),
                            stop=(j == 3),
                        )
                    nc.scalar.copy(yout[:, r, :], psum_o[:])
                nc.sync.dma_start(
                    y_d.ap()[t * super_rows : (t + 1) * super_rows, :].rearrange(
                        "(p r) e -> p r e", p=P
                    ),
                    yout[:],
                )

    nc.compile()
    return nc


VARIANT = "bf16"
SUPER_ROWS = 512


def _get_program(variant):
    key = ("prog", B_CORE, SUPER_ROWS, variant)
    if key not in _cache:
        _cache[key] = _build(B_CORE, SUPER_ROWS, variant)
    return _cache[key]


def _input_consts(variant):
    import ml_dtypes

    key = ("consts", variant)
    if key not in _cache:
        wdt = ml_dtypes.bfloat16 if variant == "bf16" else np.float32
        w = _dft_matrix_f64().astype(wdt)
        ident = np.eye(P, dtype=wdt)
        _cache[key] = (w, ident)
    return _cache[key]


def _run(x, trace=False, trace_cores=None, variant=None):
    """x: [B_TOTAL, 2, 256] f32 -> (out [B_TOTAL, 2, 256] f32, results obj)."""
    from concourse import bass_utils

    variant = variant or VARIANT
    x = np.ascontiguousarray(np.asarray(x, dtype=np.float32)).reshape(B_TOTAL, V)
    w, ident = _input_consts(variant)
    nc = _get_program(variant)
    in_maps = [
        {
            "x_in": x[c * B_CORE : (c + 1) * B_CORE],
            "w_in": w,
            "id_in": ident,
        }
        for c in range(N_CORES)
    ]
    res = bass_utils.run_bass_kernel_spmd(
        nc,
        in_maps,
        core_ids=list(range(N_CORES)),
        trace=trace,
        trace_cores=trace_cores,
    )
    out = np.concatenate([res.results[c]["y_out"] for c in range(N_CORES)], axis=0)
    return out.reshape(B_TOTAL, 2, NFFT).astype(np.float32, copy=False), res


def kernel(x):
    out, _ = _run(x, trace=False)
    return out


# revision 11
# speedup vs baseline: 1.3006x; 1.1156x over previous
"""256-point FFT (real/imag channels) as a DFT matmul on Trainium2.

Contract: kernel(x) takes the FULL input x [131072, 2, 256] float32 and
returns the FULL output [131072, 2, 256] float32, computing, per batch row,
the 256-point complex FFT of (x[b,0,:] + i*x[b,1,:]) -> [real; imag].

Strategy (pure data parallel over 8 NeuronCores, 16384 rows/core):
  - Flatten each row to v[512] = [re(256), im(256)].  The FFT is the linear
    map y = v @ W with W[512,512] built from cos/sin of the DFT twiddles.
  - Per core, stream 128-row chunks: DMA rows in natural (batch-major)
    layout, transpose each [128b x 128n] block on TensorE (transpose mode,
    via identity), copy back to SBUF, then 4 accumulating matmuls against
    W (bf16 data/weights, fp32 PSUM accumulation) produce [128b x 512out]
    batch-major, which is copied to SBUF (ScalarE) and DMA'd out.
  - bf16 inputs + fp32 accumulate gives ~2e-3 relative error vs the fp32
    reference; DMA (64 MiB/core of HBM traffic) is the roofline.
"""

import numpy as np

B_TOTAL = 131072
N_CORES = 8
B_CORE = B_TOTAL // N_CORES  # 16384
NFFT = 256
V = 2 * NFFT  # 512, flattened row length
P = 128  # partitions

_cache = {}


def _dft_matrix_f64():
    """W[n, m] such that out[b, m] = sum_n v[b, n] * W[n, m]."""
    k = np.arange(NFFT, dtype=np.float64)
    theta = -2.0 * np.pi * np.outer(k, k) / NFFT  # [k, n]
    c = np.cos(theta).T  # [n, k]
    s = np.sin(theta).T  # [n, k]
    w = np.zeros((V, V), np.float64)
    w[:NFFT, :NFFT] = c
    w[NFFT:, :NFFT] = -s
    w[:NFFT, NFFT:] = s
    w[NFFT:, NFFT:] = c
    return w


def _build(b_core, super_rows, variant="bf16"):
    """Build + compile the per-core Bass program. Returns nc.

    variant: "bf16" (cast input to bf16 during load; ~2e-3 rel err) or
    "f32r" (fp32-width data, PE fp32r decomposition; higher precision).
    """
    import concourse.bass as bass
    import concourse.tile as tile
    from concourse import bacc, mybir

    n_super = b_core // super_rows
    r_sub = super_rows // P  # 128-row sub-chunks per DMA super-chunk
    f32 = mybir.dt.float32
    cdt = mybir.dt.bfloat16 if variant == "bf16" else mybir.dt.float32r

    nc = bacc.Bacc(
        "TRN2",
        target_bir_lowering=False,
        debug=False,
        num_devices=N_CORES,
    )
    x_dt = f32 if variant == "bf16" else mybir.dt.float32r
    x_d = nc.dram_tensor("x_in", [b_core, V], x_dt, kind="ExternalInput")
    w_d = nc.dram_tensor("w_in", [V, V], cdt, kind="ExternalInput")
    id_d = nc.dram_tensor("id_in", [P, P], cdt, kind="ExternalInput")
    y_d = nc.dram_tensor("y_out", [b_core, V], f32, kind="ExternalOutput")

    with tile.TileContext(nc) as tc:
        with (
            tc.tile_pool(name="const", bufs=1) as cpool,
            tc.tile_pool(name="xin", bufs=3) as xpool,
            tc.tile_pool(name="xt", bufs=4) as xtpool,
            tc.tile_pool(name="yout", bufs=3) as ypool,
            tc.tile_pool(name="psumT", bufs=3, space="PSUM") as ptpool,
            tc.tile_pool(name="psumO", bufs=3, space="PSUM") as popool,
        ):
            w_sb = cpool.tile([P, 4, V], cdt)
            nc.sync.dma_start(w_sb[:], w_d.ap().rearrange("(j p) m -> p j m", p=P))
            id_sb = cpool.tile([P, P], cdt)
            nc.sync.dma_start(id_sb[:], id_d.ap())

            for t in range(n_super):
                # Map r_sub *consecutive* DRAM rows to each partition so every
                # partition's slice of the transfer is one contiguous run
                # (vs 2 KiB descriptors with row-round-robin layout).
                xin = xpool.tile([P, r_sub, V], cdt)
                load_eng = nc.gpsimd if variant == "bf16" else nc.sync
                x_src = x_d.ap()[t * super_rows : (t + 1) * super_rows, :].rearrange(
                    "(p r) e -> p r e", p=P
                )
                # Split the very first load so the compute pipeline fills
                # after ~1/4 of a super-chunk instead of a whole one.
                n_load = 4 if t == 0 else 1
                step = r_sub // n_load
                for h in range(n_load):
                    load_eng.dma_start(
                        xin[:, h * step : (h + 1) * step, :],
                        x_src[:, h * step : (h + 1) * step, :],
                    )
                yout = ypool.tile([P, r_sub, V], f32)
                for r in range(r_sub):
                    psum_t = ptpool.tile([P, V], cdt)
                    for j in range(4):
                        nc.tensor.transpose(
                            psum_t[:, j * P : (j + 1) * P],
                            xin[:, r, j * P : (j + 1) * P],
                            id_sb[:],
                        )
                    xt = xtpool.tile([P, V], cdt)
                    nc.vector.tensor_copy(xt[:], psum_t[:])
                    psum_o = popool.tile([P, V], f32)
                    for j in range(4):
                        nc.tensor.matmul(
                            psum_o[:],
                            xt[:, j * P : (j + 1) * P],
                            w_sb[:, j, :],
                            start=(j == 0),
                            stop=(j == 3),
                        )
                    nc.scalar.copy(yout[:, r, :], psum_o[:])
                y_dst = y_d.ap()[t * super_rows : (t + 1) * super_rows, :].rearrange(
                    "(p r) e -> p r e", p=P
                )
                # Split the very last store so the tail drain overlaps the
                # final sub-chunks' compute instead of waiting for all of it.
                n_store = 4 if t == n_super - 1 else 1
                step = r_sub // n_store
                for h in range(n_store):
                    nc.sync.dma_start(
                        y_dst[:, h * step : (h + 1) * step, :],
                        yout[:, h * step : (h + 1) * step, :],
                    )

    nc.compile()
    return nc


VARIANT = "bf16"
SUPER_ROWS = 1024


def _get_program(variant):
    key = ("prog", B_CORE, SUPER_ROWS, variant)
    if key not in _cache:
        _cache[key] = _build(B_CORE, SUPER_ROWS, variant)
    return _cache[key]


def _input_consts(variant):
    import ml_dtypes

    key = ("consts", variant)
    if key not in _cache:
        wdt = ml_dtypes.bfloat16 if variant == "bf16" else np.float32
        w = _dft_matrix_f64().astype(wdt)
        ident = np.eye(P, dtype=wdt)
        _cache[key] = (w, ident)
    return _cache[key]


def _run(x, trace=False, trace_cores=None, variant=None):
    """x: [B_TOTAL, 2, 256] f32 -> (out [B_TOTAL, 2, 256] f32, results obj)."""
    from concourse import bass_utils

    variant = variant or VARIANT
    x = np.ascontiguousarray(np.asarray(x, dtype=np.float32)).reshape(B_TOTAL, V)
    w, ident = _input_consts(variant)
    nc = _get_program(variant)
    in_maps = [
        {
            "x_in": x[c * B_CORE : (c + 1) * B_CORE],
            "w_in": w,
            "id_in": ident,
        }
        for c in range(N_CORES)
    ]
    res = bass_utils.run_bass_kernel_spmd(
        nc,
        in_maps,
        core_ids=list(range(N_CORES)),
        trace=trace,
        trace_cores=trace_cores,
    )
    out = np.concatenate([res.results[c]["y_out"] for c in range(N_CORES)], axis=0)
    return out.reshape(B_TOTAL, 2, NFFT).astype(np.float32, copy=False), res


def kernel(x):
    out, _ = _run(x, trace=False)
    return out
